# revision 23
# baseline (speedup 1.0000x reference)
"""MoE (MiMoV2) expert-parallel Trainium2 kernel.

Reference computation (T=4096, D=2048, E=32, F=1024, top-4):
  router = softmax(x @ gate_w); topk -> renormalized combine weights
  out = sum_e combine[:, e] * (silu(x @ wg_e) * (x @ wu_e)) @ wd_e

Strategy (expert parallel, ep=8):
  - Host: route tokens (CPU), gather per-expert token batches, 4 expert
    slots per core. Experts are rank-sorted by token count and distributed
    round-robin over cores, so slot j has the same capacity on every core
    (SPMD: one program for all 8 cores) and padding waste is minimized.
  - Device: per slot compute hT = silu(wg^T x^T) * (wu^T x^T) then
    yT = wd^T hT, all in transposed layout so no on-device transposes are
    needed and all weights are used in their native layout.
  - Host: scatter-add weighted expert outputs back to [T, D].

Only the top-4 experts per token are computed (the reference multiplies the
other 28 experts by an exactly-zero combine weight), so this is ~8x less
compute than the dense reference.
"""

import os

import numpy as np
import ml_dtypes

import concourse.bacc as bacc
import concourse.mybir as mybir
from concourse.tile import TileContext
from concourse.bass_utils import run_bass_kernel_spmd

P = 128
TOP_K = 4
N_CORES = 8

# Set by kernel() when KERNEL_TRACE=1: exec_time_ns of the slowest traced core.
LAST_EXEC_NS = None
LAST_RESULTS = None

_KERNEL_CACHE = {}

# matmul input dtype: "bf16" | "f32r" | "f32"
MM_DTYPE = os.environ.get("KERNEL_MM_DTYPE", "bf16")


def _mm_dt():
    return {
        "bf16": mybir.dt.bfloat16,
        "f32r": mybir.dt.float32r,
        "f32": mybir.dt.float32,
    }[MM_DTYPE]


def _np_dt():
    return {"bf16": ml_dtypes.bfloat16, "f32r": np.float32, "f32": np.float32}[
        MM_DTYPE
    ]


def _chunks(C):
    """Split C into equal-ish chunks of at most 512 columns each.

    Balanced splits ([288,288] rather than [512,64]) dodge the ~54ns
    small-matmul issue floor on the tensor engine."""
    if C <= 512:
        return [(0, C)]
    nch = -(-C // 512)
    base = -(-(-(-C // nch)) // 2) * 2
    out = []
    s = 0
    while s < C:
        w = min(base, C - s)
        out.append((s, w))
        s += w
    return out


def build_moe_kernel(caps, D, F, dt_in):
    """Bass program: per core, len(caps) expert slots; slot j holds up to
    caps[j] tokens (caps identical across cores; data differs).

    Inputs (per core, Cj = caps[j]):
      xT{j}: [P, D//P, Cj]      xT[pi, ko, c] = x_j[c, ko*P+pi]
      wg: [S, F//P, P, D//P, P] wg[j, f, pi, ko, fi] = Wg_j[ko*P+pi, f*P+fi]
      wu: same layout as wg
      wd: [S, D//P, P, F//P, P] wd[j, d, pi, ko, di] = Wd_j[ko*P+pi, d*P+di]
    Output:
      yT{j}: [D//P, P, Cj] f32  yT[d, di, c] = y_j[c, d*P+di]
    """
    KD = D // P
    KF = F // P
    S = len(caps)
    f32 = mybir.dt.float32
    nc = bacc.Bacc(None)

    xTs = [
        nc.declare_dram_parameter(f"xT{j}", [P, KD, c], dt_in, isOutput=False)
        for j, c in enumerate(caps)
    ]
    wg = nc.declare_dram_parameter("wg", [S, KF, P, KD, P], dt_in, isOutput=False)
    wu = nc.declare_dram_parameter("wu", [S, KF, P, KD, P], dt_in, isOutput=False)
    wd = nc.declare_dram_parameter("wd", [S, KD, P, KF, P], dt_in, isOutput=False)
    yTs = [
        nc.declare_dram_parameter(f"yT{j}", [KD, P, c], dt_in, isOutput=True)
        for j, c in enumerate(caps)
    ]

    with TileContext(nc) as tc:
        XB = 4  # k-tiles per x-load batch
        wide = dt_in != mybir.dt.bfloat16  # 4-byte dtypes: halve prefetch depth
        bA, bB, bY, bT = (3, 3, 2, 2) if wide else (8, 6, 3, 3)
        with (
            tc.tile_pool(name="xp", bufs=2) as xp,
            tc.tile_pool(name="wA", bufs=bA) as wA,
            tc.tile_pool(name="wB", bufs=bB) as wB,
            tc.tile_pool(name="hp", bufs=3) as hp,
            tc.tile_pool(name="tp", bufs=bT) as tp,
            tc.tile_pool(name="yp", bufs=bY) as yp,
            tc.tile_pool(name="psA", bufs=2, space="PSUM") as psA,
            tc.tile_pool(name="psB", bufs=4, space="PSUM") as psB,
        ):
            # Warmup: ~32 junk matmuls with no DMA dependency fill the
            # initial input-DMA window and un-throttle the PE HAM clock
            # gate (cold 1.2GHz -> warm 2.4GHz) before real work arrives.
            warm = tp.tile([P, P], mybir.dt.bfloat16, tag="warm")
            nc.any.memset(warm[:], 0.0)
            pw = psA.tile([P, P], f32, tag="pg")
            for _ in range(32):
                nc.tensor.matmul(pw[:], lhsT=warm[:], rhs=warm[:], start=True, stop=True)

            for j, C in enumerate(caps):
                cks = _chunks(C)
                # Issue f0 weight loads before the x tiles: DMA descriptor
                # issues serialize at ~650ns each on the sync sequencer, and
                # the first matmul needs wg[f0] + x[k0] only.
                wg_f0 = wA.tile([P, KD, P], dt_in, tag="wg")
                nc.sync.dma_start(wg_f0[:], wg[j, 0])
                # x tiles batched XB k-slices per DMA: the first matmul only
                # waits on the first batch, without paying 16 issue slots
                x_sb = []
                for k0 in range(0, KD, XB):
                    xk = xp.tile([P, XB, C], dt_in, tag=f"x{k0}")
                    nc.sync.dma_start(xk[:], xTs[j][:, k0 : k0 + XB, :])
                    x_sb.append(xk)
                    if k0 == 0:
                        # wu is first needed a full k-loop (~2us) after wg
                        wu_f0 = wA.tile([P, KD, P], dt_in, tag="wu")
                        nc.sync.dma_start(wu_f0[:], wu[j, 0])
                h_sb = hp.tile([P, KF, C], dt_in, tag="h")

                # Phase A: hT[f*P+pi, c] = silu(gT) * uT
                for f in range(KF):
                    if f == 0:
                        wg_sb, wu_sb = wg_f0, wu_f0
                    else:
                        wg_sb = wA.tile([P, KD, P], dt_in, tag="wg")
                        nc.sync.dma_start(wg_sb[:], wg[j, f])
                        wu_sb = wA.tile([P, KD, P], dt_in, tag="wu")
                        nc.sync.dma_start(wu_sb[:], wu[j, f])
                    for c0, cw in cks:
                        pg = psA.tile([P, cw], f32, tag="pg")
                        for k in range(KD):
                            nc.tensor.matmul(
                                pg[:],
                                lhsT=wg_sb[:, k, :],
                                rhs=x_sb[k // XB][:, k % XB, c0 : c0 + cw],
                                start=(k == 0),
                                stop=(k == KD - 1),
                            )
                        pu = psA.tile([P, cw], f32, tag="pu")
                        for k in range(KD):
                            nc.tensor.matmul(
                                pu[:],
                                lhsT=wu_sb[:, k, :],
                                rhs=x_sb[k // XB][:, k % XB, c0 : c0 + cw],
                                start=(k == 0),
                                stop=(k == KD - 1),
                            )
                        tmp = tp.tile([P, cw], f32, tag="tmp")
                        nc.scalar.activation(
                            tmp[:], pg[:], mybir.ActivationFunctionType.Silu
                        )
                        nc.vector.tensor_mul(
                            out=h_sb[:, f, c0 : c0 + cw], in0=tmp[:], in1=pu[:]
                        )

                # Phase B: yT[d*P+di, c] = wd^T @ hT
                # y stores batched per d-tile pair to halve DMA issue count
                for d in range(KD):
                    wd_sb = wB.tile([P, KF, P], dt_in, tag="wd")
                    nc.sync.dma_start(wd_sb[:], wd[j, d])
                    if d % 2 == 0:
                        y_sb = yp.tile([P, 2, C], dt_in, tag="y")
                    for c0, cw in cks:
                        py = psB.tile([P, cw], f32, tag="py")
                        for k in range(KF):
                            nc.tensor.matmul(
                                py[:],
                                lhsT=wd_sb[:, k, :],
                                rhs=h_sb[:, k, c0 : c0 + cw],
                                start=(k == 0),
                                stop=(k == KF - 1),
                            )
                        nc.vector.tensor_copy(y_sb[:, d % 2, c0 : c0 + cw], py[:])
                    if d % 2 == 1:
                        nc.sync.dma_start(
                            yTs[j][d - 1 : d + 1].rearrange("d p c -> p d c"),
                            y_sb[:],
                        )
    nc.compile()
    return nc


def _route(hidden_states, gate_w):
    """Router: same math as the reference (softmax then top-k, renormalize).

    Runs in float32 numpy; top-k order matches lax.top_k (descending, ties
    broken by lower index)."""
    logits = (hidden_states.astype(np.float32) @ gate_w.astype(np.float32)).astype(
        np.float32
    )
    m = logits.max(axis=-1, keepdims=True)
    ex = np.exp(logits - m, dtype=np.float32)
    probs = ex / ex.sum(axis=-1, keepdims=True, dtype=np.float32)
    # descending stable sort == lax.top_k tie semantics
    order = np.argsort(-probs, axis=-1, kind="stable")
    topk_ids = order[:, :TOP_K].astype(np.int32)
    topk_w = np.take_along_axis(probs, topk_ids, axis=-1)
    topk_w = topk_w / topk_w.sum(axis=-1, keepdims=True)
    return topk_ids, topk_w.astype(np.float32)


def _pack_weight(w):
    """[E, A, B] -> per-expert SBUF-layout tiles [E, B//P, P(a_i), A//P, P(b_i)]
    with element (e, b_o, a_i, a_o, b_i) = w[e, a_o*P+a_i, b_o*P+b_i]."""
    E, A, B = w.shape
    return np.ascontiguousarray(
        w.reshape(E, A // P, P, B // P, P).transpose(0, 3, 2, 1, 4)
    )


def kernel(hidden_states, gate_w, w_gate, w_up, w_down):
    global LAST_EXEC_NS, LAST_RESULTS
    T, D = hidden_states.shape
    E = gate_w.shape[1]
    F = w_gate.shape[2]
    S = E // N_CORES  # expert slots per core

    hidden_states = np.asarray(hidden_states)
    gate_w = np.asarray(gate_w)
    w_gate = np.asarray(w_gate)
    w_up = np.asarray(w_up)
    w_down = np.asarray(w_down)

    topk_ids, topk_w = _route(hidden_states, gate_w)

    # token lists per expert
    tok_idx = []
    wts = []
    for e in range(E):
        t_mask, k_pos = np.nonzero(topk_ids == e)
        tok_idx.append(t_mask.astype(np.int64))
        wts.append(topk_w[t_mask, k_pos])
    counts = np.array([len(ix) for ix in tok_idx])

    # rank-sort experts; rank r -> core r % 8, slot r // 8. caps[j] is the
    # max count among slot-j experts, rounded up to a multiple of 32.
    order = np.argsort(-counts, kind="stable")
    slot_expert = [
        [int(order[j * N_CORES + c]) for c in range(N_CORES)] for j in range(S)
    ]
    caps = tuple(
        max(32, int(-(-max(counts[e] for e in slot) // 2) * 2))
        for slot in slot_expert
    )

    np_dt = _np_dt()
    dt_in = _mm_dt()
    KD, KF = D // P, F // P

    wgp = _pack_weight(w_gate.astype(np_dt))
    wup = _pack_weight(w_up.astype(np_dt))
    wdp = _pack_weight(w_down.astype(np_dt))

    in_maps = []
    for core in range(N_CORES):
        m = {}
        sel = [slot_expert[j][core] for j in range(S)]
        m["wg"] = np.ascontiguousarray(wgp[sel])
        m["wu"] = np.ascontiguousarray(wup[sel])
        m["wd"] = np.ascontiguousarray(wdp[sel])
        for j in range(S):
            e = sel[j]
            C = caps[j]
            ix = tok_idx[e]
            xTc = np.zeros((P, KD, C), dtype=np_dt)
            if len(ix):
                xe = hidden_states[ix].astype(np_dt)  # [n, D]
                xTc[:, :, : len(ix)] = xe.T.reshape(KD, P, len(ix)).transpose(
                    1, 0, 2
                )
            m[f"xT{j}"] = xTc
        in_maps.append(m)

    if os.environ.get("KERNEL_TRACE", "0") == "1":
        import sys

        print(f"[kernel] counts max={counts.max()} caps={caps}", file=sys.stderr)

    key = (caps, D, F, MM_DTYPE)
    if key not in _KERNEL_CACHE:
        _KERNEL_CACHE[key] = build_moe_kernel(caps, D, F, dt_in)
    nc = _KERNEL_CACHE[key]

    trace = os.environ.get("KERNEL_TRACE", "0") == "1"
    res = run_bass_kernel_spmd(
        nc,
        in_maps,
        list(range(N_CORES)),
        trace=trace,
    )
    LAST_EXEC_NS = res.exec_time_ns
    LAST_RESULTS = res

    out = np.zeros((T, D), dtype=np.float32)
    for core in range(N_CORES):
        for j in range(S):
            e = slot_expert[j][core]
            ix = tok_idx[e]
            if len(ix) == 0:
                continue
            yT = res.results[core][f"yT{j}"].astype(np.float32)  # [KD, P, C]
            ye = yT[:, :, : len(ix)].transpose(2, 0, 1).reshape(len(ix), D)
            out[ix] += wts[e][:, None] * ye
    return out.astype(hidden_states.dtype), topk_ids


# revision 24
# speedup vs baseline: 1.0068x; 1.0068x over previous
"""MoE (MiMoV2) expert-parallel Trainium2 kernel.

Reference computation (T=4096, D=2048, E=32, F=1024, top-4):
  router = softmax(x @ gate_w); topk -> renormalized combine weights
  out = sum_e combine[:, e] * (silu(x @ wg_e) * (x @ wu_e)) @ wd_e

Strategy (expert parallel, ep=8):
  - Host: route tokens (CPU), gather per-expert token batches, 4 expert
    slots per core. Experts are rank-sorted by token count and distributed
    round-robin over cores, so slot j has the same capacity on every core
    (SPMD: one program for all 8 cores) and padding waste is minimized.
  - Device: per slot compute hT = silu(wg^T x^T) * (wu^T x^T) then
    yT = wd^T hT, all in transposed layout so no on-device transposes are
    needed and all weights are used in their native layout.
  - Host: scatter-add weighted expert outputs back to [T, D].

Only the top-4 experts per token are computed (the reference multiplies the
other 28 experts by an exactly-zero combine weight), so this is ~8x less
compute than the dense reference.
"""

import os

import numpy as np
import ml_dtypes

import concourse.bacc as bacc
import concourse.mybir as mybir
from concourse.tile import TileContext
from concourse.bass_utils import run_bass_kernel_spmd

P = 128
TOP_K = 4
N_CORES = 8

# Set by kernel() when KERNEL_TRACE=1: exec_time_ns of the slowest traced core.
LAST_EXEC_NS = None
LAST_RESULTS = None

_KERNEL_CACHE = {}

# matmul input dtype: "bf16" | "f32r" | "f32"
MM_DTYPE = os.environ.get("KERNEL_MM_DTYPE", "bf16")


def _mm_dt():
    return {
        "bf16": mybir.dt.bfloat16,
        "f32r": mybir.dt.float32r,
        "f32": mybir.dt.float32,
    }[MM_DTYPE]


def _np_dt():
    return {"bf16": ml_dtypes.bfloat16, "f32r": np.float32, "f32": np.float32}[
        MM_DTYPE
    ]


def _chunks(C):
    """Split C into equal-ish chunks of at most 512 columns each.

    Balanced splits ([288,288] rather than [512,64]) dodge the ~54ns
    small-matmul issue floor on the tensor engine."""
    if C <= 512:
        return [(0, C)]
    nch = -(-C // 512)
    base = -(-(-(-C // nch)) // 8) * 8
    out = []
    s = 0
    while s < C:
        w = min(base, C - s)
        out.append((s, w))
        s += w
    return out


def build_moe_kernel(caps, D, F, dt_in):
    """Bass program: per core, len(caps) expert slots; slot j holds up to
    caps[j] tokens (caps identical across cores; data differs).

    Inputs (per core, Cj = caps[j]):
      xT{j}: [P, D//P, Cj]      xT[pi, ko, c] = x_j[c, ko*P+pi]
      wg: [S, F//P, P, D//P, P] wg[j, f, pi, ko, fi] = Wg_j[ko*P+pi, f*P+fi]
      wu: same layout as wg
      wd: [S, D//P, P, F//P, P] wd[j, d, pi, ko, di] = Wd_j[ko*P+pi, d*P+di]
    Output:
      yT{j}: [D//P, P, Cj] f32  yT[d, di, c] = y_j[c, d*P+di]
    """
    KD = D // P
    KF = F // P
    S = len(caps)
    f32 = mybir.dt.float32
    nc = bacc.Bacc(None)

    xTs = [
        nc.declare_dram_parameter(f"xT{j}", [P, KD, c], dt_in, isOutput=False)
        for j, c in enumerate(caps)
    ]
    wg = nc.declare_dram_parameter("wg", [S, KF, P, KD, P], dt_in, isOutput=False)
    wu = nc.declare_dram_parameter("wu", [S, KF, P, KD, P], dt_in, isOutput=False)
    wd = nc.declare_dram_parameter("wd", [S, KD, P, KF, P], dt_in, isOutput=False)
    yTs = [
        nc.declare_dram_parameter(f"yT{j}", [KD, P, c], dt_in, isOutput=True)
        for j, c in enumerate(caps)
    ]

    with TileContext(nc) as tc:
        XB = 4  # k-tiles per x-load batch
        wide = dt_in != mybir.dt.bfloat16  # 4-byte dtypes: halve prefetch depth
        bA, bB, bY, bT = (3, 3, 2, 2) if wide else (6, 6, 3, 3)
        with (
            tc.tile_pool(name="xp", bufs=2) as xp,
            tc.tile_pool(name="wA", bufs=bA) as wA,
            tc.tile_pool(name="wB", bufs=bB) as wB,
            tc.tile_pool(name="hp", bufs=2) as hp,
            tc.tile_pool(name="tp", bufs=bT) as tp,
            tc.tile_pool(name="yp", bufs=bY) as yp,
            tc.tile_pool(name="psA", bufs=2, space="PSUM") as psA,
            tc.tile_pool(name="psB", bufs=4, space="PSUM") as psB,
        ):
            # Warmup: ~32 junk matmuls with no DMA dependency fill the
            # initial input-DMA window and un-throttle the PE HAM clock
            # gate (cold 1.2GHz -> warm 2.4GHz) before real work arrives.
            warm = tp.tile([P, P], mybir.dt.bfloat16, tag="warm")
            nc.any.memset(warm[:], 0.0)
            pw = psA.tile([P, P], f32, tag="pg")
            for _ in range(32):
                nc.tensor.matmul(pw[:], lhsT=warm[:], rhs=warm[:], start=True, stop=True)

            for j, C in enumerate(caps):
                cks = _chunks(C)
                # Issue f0 weight loads before the x tiles: DMA descriptor
                # issues serialize at ~650ns each on the sync sequencer, and
                # the first matmul needs wg[f0] + x[k0] only.
                wg_f0 = wA.tile([P, KD, P], dt_in, tag="wg")
                nc.sync.dma_start(wg_f0[:], wg[j, 0])
                # x tiles batched XB k-slices per DMA: the first matmul only
                # waits on the first batch, without paying 16 issue slots
                x_sb = []
                for k0 in range(0, KD, XB):
                    xk = xp.tile([P, XB, C], dt_in, tag=f"x{k0}")
                    nc.sync.dma_start(xk[:], xTs[j][:, k0 : k0 + XB, :])
                    x_sb.append(xk)
                    if k0 == 0:
                        # wu is first needed a full k-loop (~2us) after wg
                        wu_f0 = wA.tile([P, KD, P], dt_in, tag="wu")
                        nc.sync.dma_start(wu_f0[:], wu[j, 0])
                h_sb = hp.tile([P, KF, C], dt_in, tag="h")

                # Phase A: hT[f*P+pi, c] = silu(gT) * uT
                for f in range(KF):
                    if f == 0:
                        wg_sb, wu_sb = wg_f0, wu_f0
                    else:
                        wg_sb = wA.tile([P, KD, P], dt_in, tag="wg")
                        nc.sync.dma_start(wg_sb[:], wg[j, f])
                        wu_sb = wA.tile([P, KD, P], dt_in, tag="wu")
                        nc.sync.dma_start(wu_sb[:], wu[j, f])
                    for c0, cw in cks:
                        pg = psA.tile([P, cw], f32, tag="pg")
                        for k in range(KD):
                            nc.tensor.matmul(
                                pg[:],
                                lhsT=wg_sb[:, k, :],
                                rhs=x_sb[k // XB][:, k % XB, c0 : c0 + cw],
                                start=(k == 0),
                                stop=(k == KD - 1),
                            )
                        pu = psA.tile([P, cw], f32, tag="pu")
                        for k in range(KD):
                            nc.tensor.matmul(
                                pu[:],
                                lhsT=wu_sb[:, k, :],
                                rhs=x_sb[k // XB][:, k % XB, c0 : c0 + cw],
                                start=(k == 0),
                                stop=(k == KD - 1),
                            )
                        tmp = tp.tile([P, cw], f32, tag="tmp")
                        nc.scalar.activation(
                            tmp[:], pg[:], mybir.ActivationFunctionType.Silu
                        )
                        nc.vector.tensor_mul(
                            out=h_sb[:, f, c0 : c0 + cw], in0=tmp[:], in1=pu[:]
                        )

                # Phase B: yT[d*P+di, c] = wd^T @ hT
                # y stores batched per d-tile pair to halve DMA issue count
                for d in range(KD):
                    wd_sb = wB.tile([P, KF, P], dt_in, tag="wd")
                    nc.sync.dma_start(wd_sb[:], wd[j, d])
                    if d % 2 == 0:
                        y_sb = yp.tile([P, 2, C], dt_in, tag="y")
                    for c0, cw in cks:
                        py = psB.tile([P, cw], f32, tag="py")
                        for k in range(KF):
                            nc.tensor.matmul(
                                py[:],
                                lhsT=wd_sb[:, k, :],
                                rhs=h_sb[:, k, c0 : c0 + cw],
                                start=(k == 0),
                                stop=(k == KF - 1),
                            )
                        nc.vector.tensor_copy(y_sb[:, d % 2, c0 : c0 + cw], py[:])
                    if d % 2 == 1:
                        nc.sync.dma_start(
                            yTs[j][d - 1 : d + 1].rearrange("d p c -> p d c"),
                            y_sb[:],
                        )
    nc.compile()
    return nc


def _route(hidden_states, gate_w):
    """Router: same math as the reference (softmax then top-k, renormalize).

    Runs in float32 numpy; top-k order matches lax.top_k (descending, ties
    broken by lower index)."""
    logits = (hidden_states.astype(np.float32) @ gate_w.astype(np.float32)).astype(
        np.float32
    )
    m = logits.max(axis=-1, keepdims=True)
    ex = np.exp(logits - m, dtype=np.float32)
    probs = ex / ex.sum(axis=-1, keepdims=True, dtype=np.float32)
    # descending stable sort == lax.top_k tie semantics
    order = np.argsort(-probs, axis=-1, kind="stable")
    topk_ids = order[:, :TOP_K].astype(np.int32)
    topk_w = np.take_along_axis(probs, topk_ids, axis=-1)
    topk_w = topk_w / topk_w.sum(axis=-1, keepdims=True)
    return topk_ids, topk_w.astype(np.float32)


def _pack_weight(w):
    """[E, A, B] -> per-expert SBUF-layout tiles [E, B//P, P(a_i), A//P, P(b_i)]
    with element (e, b_o, a_i, a_o, b_i) = w[e, a_o*P+a_i, b_o*P+b_i]."""
    E, A, B = w.shape
    return np.ascontiguousarray(
        w.reshape(E, A // P, P, B // P, P).transpose(0, 3, 2, 1, 4)
    )


def kernel(hidden_states, gate_w, w_gate, w_up, w_down):
    global LAST_EXEC_NS, LAST_RESULTS
    T, D = hidden_states.shape
    E = gate_w.shape[1]
    F = w_gate.shape[2]
    S = E // N_CORES  # expert slots per core

    hidden_states = np.asarray(hidden_states)
    gate_w = np.asarray(gate_w)
    w_gate = np.asarray(w_gate)
    w_up = np.asarray(w_up)
    w_down = np.asarray(w_down)

    topk_ids, topk_w = _route(hidden_states, gate_w)

    # token lists per expert
    tok_idx = []
    wts = []
    for e in range(E):
        t_mask, k_pos = np.nonzero(topk_ids == e)
        tok_idx.append(t_mask.astype(np.int64))
        wts.append(topk_w[t_mask, k_pos])
    counts = np.array([len(ix) for ix in tok_idx])

    # rank-sort experts; rank r -> core r % 8, slot r // 8. caps[j] is the
    # max count among slot-j experts, rounded up to a multiple of 32.
    order = np.argsort(-counts, kind="stable")
    slot_expert = [
        [int(order[j * N_CORES + c]) for c in range(N_CORES)] for j in range(S)
    ]
    caps = tuple(
        max(32, int(-(-max(counts[e] for e in slot) // 8) * 8))
        for slot in slot_expert
    )

    np_dt = _np_dt()
    dt_in = _mm_dt()
    KD, KF = D // P, F // P

    wgp = _pack_weight(w_gate.astype(np_dt))
    wup = _pack_weight(w_up.astype(np_dt))
    wdp = _pack_weight(w_down.astype(np_dt))

    in_maps = []
    for core in range(N_CORES):
        m = {}
        sel = [slot_expert[j][core] for j in range(S)]
        m["wg"] = np.ascontiguousarray(wgp[sel])
        m["wu"] = np.ascontiguousarray(wup[sel])
        m["wd"] = np.ascontiguousarray(wdp[sel])
        for j in range(S):
            e = sel[j]
            C = caps[j]
            ix = tok_idx[e]
            xTc = np.zeros((P, KD, C), dtype=np_dt)
            if len(ix):
                xe = hidden_states[ix].astype(np_dt)  # [n, D]
                xTc[:, :, : len(ix)] = xe.T.reshape(KD, P, len(ix)).transpose(
                    1, 0, 2
                )
            m[f"xT{j}"] = xTc
        in_maps.append(m)

    if os.environ.get("KERNEL_TRACE", "0") == "1":
        import sys

        print(f"[kernel] counts max={counts.max()} caps={caps}", file=sys.stderr)

    key = (caps, D, F, MM_DTYPE)
    if key not in _KERNEL_CACHE:
        _KERNEL_CACHE[key] = build_moe_kernel(caps, D, F, dt_in)
    nc = _KERNEL_CACHE[key]

    trace = os.environ.get("KERNEL_TRACE", "0") == "1"
    res = run_bass_kernel_spmd(
        nc,
        in_maps,
        list(range(N_CORES)),
        trace=trace,
    )
    LAST_EXEC_NS = res.exec_time_ns
    LAST_RESULTS = res

    out = np.zeros((T, D), dtype=np.float32)
    for core in range(N_CORES):
        for j in range(S):
            e = slot_expert[j][core]
            ix = tok_idx[e]
            if len(ix) == 0:
                continue
            yT = res.results[core][f"yT{j}"].astype(np.float32)  # [KD, P, C]
            ye = yT[:, :, : len(ix)].transpose(2, 0, 1).reshape(len(ix), D)
            out[ix] += wts[e][:, None] * ye
    return out.astype(hidden_states.dtype), topk_ids


# revision 26
# speedup vs baseline: 1.0109x; 1.0040x over previous
"""MoE (MiMoV2) expert-parallel Trainium2 kernel.

Reference computation (T=4096, D=2048, E=32, F=1024, top-4):
  router = softmax(x @ gate_w); topk -> renormalized combine weights
  out = sum_e combine[:, e] * (silu(x @ wg_e) * (x @ wu_e)) @ wd_e

Strategy (expert parallel, ep=8):
  - Host: route tokens (CPU), gather per-expert token batches, 4 expert
    slots per core. Experts are rank-sorted by token count and distributed
    round-robin over cores, so slot j has the same capacity on every core
    (SPMD: one program for all 8 cores) and padding waste is minimized.
  - Device: per slot compute hT = silu(wg^T x^T) * (wu^T x^T) then
    yT = wd^T hT, all in transposed layout so no on-device transposes are
    needed and all weights are used in their native layout.
  - Host: scatter-add weighted expert outputs back to [T, D].

Only the top-4 experts per token are computed (the reference multiplies the
other 28 experts by an exactly-zero combine weight), so this is ~8x less
compute than the dense reference.
"""

import os

import numpy as np
import ml_dtypes

import concourse.bacc as bacc
import concourse.mybir as mybir
from concourse.tile import TileContext
from concourse.bass_utils import run_bass_kernel_spmd

P = 128
TOP_K = 4
N_CORES = 8

# Set by kernel() when KERNEL_TRACE=1: exec_time_ns of the slowest traced core.
LAST_EXEC_NS = None
LAST_RESULTS = None

_KERNEL_CACHE = {}

# matmul input dtype: "bf16" | "f32r" | "f32"
MM_DTYPE = os.environ.get("KERNEL_MM_DTYPE", "bf16")


def _mm_dt():
    return {
        "bf16": mybir.dt.bfloat16,
        "f32r": mybir.dt.float32r,
        "f32": mybir.dt.float32,
    }[MM_DTYPE]


def _np_dt():
    return {"bf16": ml_dtypes.bfloat16, "f32r": np.float32, "f32": np.float32}[
        MM_DTYPE
    ]


def _chunks(C):
    """Split C into equal-ish chunks of at most 512 columns each.

    Balanced splits ([288,288] rather than [512,64]) dodge the ~54ns
    small-matmul issue floor on the tensor engine."""
    if C <= 512:
        return [(0, C)]
    nch = -(-C // 512)
    base = -(-(-(-C // nch)) // 8) * 8
    out = []
    s = 0
    while s < C:
        w = min(base, C - s)
        out.append((s, w))
        s += w
    return out


def build_moe_kernel(caps, D, F, dt_in):
    """Bass program: per core, len(caps) expert slots; slot j holds up to
    caps[j] tokens (caps identical across cores; data differs).

    Inputs (per core, Cj = caps[j]):
      xT{j}: [P, D//P, Cj]      xT[pi, ko, c] = x_j[c, ko*P+pi]
      wg: [S, F//P, P, D//P, P] wg[j, f, pi, ko, fi] = Wg_j[ko*P+pi, f*P+fi]
      wu: same layout as wg
      wd: [S, D//P, P, F//P, P] wd[j, d, pi, ko, di] = Wd_j[ko*P+pi, d*P+di]
    Output:
      yT{j}: [D//P, P, Cj] f32  yT[d, di, c] = y_j[c, d*P+di]
    """
    KD = D // P
    KF = F // P
    S = len(caps)
    f32 = mybir.dt.float32
    nc = bacc.Bacc(None)

    xTs = [
        nc.declare_dram_parameter(f"xT{j}", [P, KD, c], dt_in, isOutput=False)
        for j, c in enumerate(caps)
    ]
    wg = nc.declare_dram_parameter("wg", [S, KF, P, KD, P], dt_in, isOutput=False)
    wu = nc.declare_dram_parameter("wu", [S, KF, P, KD, P], dt_in, isOutput=False)
    wd = nc.declare_dram_parameter("wd", [S, KD, P, KF, P], dt_in, isOutput=False)
    yTs = [
        nc.declare_dram_parameter(f"yT{j}", [KD, P, c], dt_in, isOutput=True)
        for j, c in enumerate(caps)
    ]

    with TileContext(nc) as tc:
        XB = 4  # k-tiles per x-load batch
        wide = dt_in != mybir.dt.bfloat16  # 4-byte dtypes: halve prefetch depth
        bA, bB, bY, bT = (3, 3, 2, 2) if wide else (6, 6, 3, 3)
        with (
            tc.tile_pool(name="xp", bufs=2) as xp,
            tc.tile_pool(name="wA", bufs=bA) as wA,
            tc.tile_pool(name="wB", bufs=bB) as wB,
            tc.tile_pool(name="hp", bufs=2) as hp,
            tc.tile_pool(name="tp", bufs=bT) as tp,
            tc.tile_pool(name="yp", bufs=bY) as yp,
            tc.tile_pool(name="psA", bufs=2, space="PSUM") as psA,
            tc.tile_pool(name="psB", bufs=4, space="PSUM") as psB,
        ):
            # Warmup: ~32 junk matmuls with no DMA dependency fill the
            # initial input-DMA window and un-throttle the PE HAM clock
            # gate (cold 1.2GHz -> warm 2.4GHz) before real work arrives.
            warm = tp.tile([P, P], mybir.dt.bfloat16, tag="warm")
            nc.any.memset(warm[:], 0.0)
            pw = psA.tile([P, P], f32, tag="pg")
            for _ in range(32):
                nc.tensor.matmul(pw[:], lhsT=warm[:], rhs=warm[:], start=True, stop=True)

            for j, C in enumerate(caps):
                cks = _chunks(C)
                # Issue f0 weight loads before the x tiles: DMA descriptor
                # issues serialize at ~650ns each on the sync sequencer, and
                # the first matmul needs wg[f0] + x[k0] only.
                wg_f0 = wA.tile([P, KD, P], dt_in, tag="wg")
                nc.sync.dma_start(wg_f0[:], wg[j, 0])
                # x tiles batched XB k-slices per DMA: the first matmul only
                # waits on the first batch, without paying 16 issue slots
                x_sb = []
                for k0 in range(0, KD, XB):
                    xk = xp.tile([P, XB, C], dt_in, tag=f"x{k0}")
                    nc.sync.dma_start(xk[:], xTs[j][:, k0 : k0 + XB, :])
                    x_sb.append(xk)
                    if k0 == 0:
                        # wu is first needed a full k-loop (~2us) after wg
                        wu_f0 = wA.tile([P, KD, P], dt_in, tag="wu")
                        nc.sync.dma_start(wu_f0[:], wu[j, 0])
                h_sb = hp.tile([P, KF, C], dt_in, tag="h")

                # Phase A: hT[f*P+pi, c] = silu(gT) * uT
                for f in range(KF):
                    if f == 0:
                        wg_sb, wu_sb = wg_f0, wu_f0
                    else:
                        wg_sb = wA.tile([P, KD, P], dt_in, tag="wg")
                        nc.sync.dma_start(wg_sb[:], wg[j, f])
                        wu_sb = wA.tile([P, KD, P], dt_in, tag="wu")
                        nc.sync.dma_start(wu_sb[:], wu[j, f])
                    for c0, cw in cks:
                        pg = psA.tile([P, cw], f32, tag="pg")
                        for k in range(KD):
                            nc.tensor.matmul(
                                pg[:],
                                lhsT=wg_sb[:, k, :],
                                rhs=x_sb[k // XB][:, k % XB, c0 : c0 + cw],
                                start=(k == 0),
                                stop=(k == KD - 1),
                            )
                        pu = psA.tile([P, cw], f32, tag="pu")
                        for k in range(KD):
                            nc.tensor.matmul(
                                pu[:],
                                lhsT=wu_sb[:, k, :],
                                rhs=x_sb[k // XB][:, k % XB, c0 : c0 + cw],
                                start=(k == 0),
                                stop=(k == KD - 1),
                            )
                        tmp = tp.tile([P, cw], f32, tag="tmp")
                        nc.scalar.activation(
                            tmp[:], pg[:], mybir.ActivationFunctionType.Silu
                        )
                        nc.vector.tensor_mul(
                            out=h_sb[:, f, c0 : c0 + cw], in0=tmp[:], in1=pu[:]
                        )

                # Phase B: yT[d*P+di, c] = wd^T @ hT
                # y stores batched per d-tile pair to halve DMA issue count
                for d in range(KD):
                    wd_sb = wB.tile([P, KF, P], dt_in, tag="wd")
                    nc.sync.dma_start(wd_sb[:], wd[j, d])
                    if d % 2 == 0:
                        y_sb = yp.tile([P, 2, C], dt_in, tag="y")
                    for c0, cw in cks:
                        py = psB.tile([P, cw], f32, tag="py")
                        for k in range(KF):
                            nc.tensor.matmul(
                                py[:],
                                lhsT=wd_sb[:, k, :],
                                rhs=h_sb[:, k, c0 : c0 + cw],
                                start=(k == 0),
                                stop=(k == KF - 1),
                            )
                        nc.vector.tensor_copy(y_sb[:, d % 2, c0 : c0 + cw], py[:])
                    if d % 2 == 1:
                        nc.sync.dma_start(
                            yTs[j][d - 1 : d + 1].rearrange("d p c -> p d c"),
                            y_sb[:],
                        )
    nc.compile()
    return nc


def _route(hidden_states, gate_w):
    """Router: same math as the reference (softmax then top-k, renormalize).

    Runs in float32 numpy; top-k order matches lax.top_k (descending, ties
    broken by lower index)."""
    logits = (hidden_states.astype(np.float32) @ gate_w.astype(np.float32)).astype(
        np.float32
    )
    m = logits.max(axis=-1, keepdims=True)
    ex = np.exp(logits - m, dtype=np.float32)
    probs = ex / ex.sum(axis=-1, keepdims=True, dtype=np.float32)
    # descending stable sort == lax.top_k tie semantics
    order = np.argsort(-probs, axis=-1, kind="stable")
    topk_ids = order[:, :TOP_K].astype(np.int32)
    topk_w = np.take_along_axis(probs, topk_ids, axis=-1)
    topk_w = topk_w / topk_w.sum(axis=-1, keepdims=True)
    return topk_ids, topk_w.astype(np.float32)


def _pack_weight(w):
    """[E, A, B] -> per-expert SBUF-layout tiles [E, B//P, P(a_i), A//P, P(b_i)]
    with element (e, b_o, a_i, a_o, b_i) = w[e, a_o*P+a_i, b_o*P+b_i]."""
    E, A, B = w.shape
    return np.ascontiguousarray(
        w.reshape(E, A // P, P, B // P, P).transpose(0, 3, 2, 1, 4)
    )


def kernel(hidden_states, gate_w, w_gate, w_up, w_down):
    global LAST_EXEC_NS, LAST_RESULTS
    T, D = hidden_states.shape
    E = gate_w.shape[1]
    F = w_gate.shape[2]
    S = E // N_CORES  # expert slots per core

    hidden_states = np.asarray(hidden_states)
    gate_w = np.asarray(gate_w)
    w_gate = np.asarray(w_gate)
    w_up = np.asarray(w_up)
    w_down = np.asarray(w_down)

    topk_ids, topk_w = _route(hidden_states, gate_w)

    # token lists per expert
    tok_idx = []
    wts = []
    for e in range(E):
        t_mask, k_pos = np.nonzero(topk_ids == e)
        tok_idx.append(t_mask.astype(np.int64))
        wts.append(topk_w[t_mask, k_pos])
    counts = np.array([len(ix) for ix in tok_idx])

    # rank-sort experts; rank r -> core r % 8, slot r // 8. caps[j] is the
    # max count among slot-j experts, rounded up to a multiple of 32.
    order = np.argsort(-counts, kind="stable")
    slot_expert = [
        [int(order[j * N_CORES + c]) for c in range(N_CORES)] for j in range(S)
    ]
    caps = tuple(
        max(32, int(-(-max(counts[e] for e in slot) // 8) * 8))
        for slot in slot_expert
    )

    np_dt = _np_dt()
    dt_in = _mm_dt()
    KD, KF = D // P, F // P

    wgp = _pack_weight(w_gate.astype(np_dt))
    wup = _pack_weight(w_up.astype(np_dt))
    wdp = _pack_weight(w_down.astype(np_dt))

    in_maps = []
    for core in range(N_CORES):
        m = {}
        sel = [slot_expert[j][core] for j in range(S)]
        m["wg"] = np.ascontiguousarray(wgp[sel])
        m["wu"] = np.ascontiguousarray(wup[sel])
        m["wd"] = np.ascontiguousarray(wdp[sel])
        for j in range(S):
            e = sel[j]
            C = caps[j]
            ix = tok_idx[e]
            xTc = np.zeros((P, KD, C), dtype=np_dt)
            if len(ix):
                xe = hidden_states[ix].astype(np_dt)  # [n, D]
                xTc[:, :, : len(ix)] = xe.T.reshape(KD, P, len(ix)).transpose(
                    1, 0, 2
                )
            m[f"xT{j}"] = xTc
        in_maps.append(m)

    if os.environ.get("KERNEL_TRACE", "0") == "1":
        import sys

        print(f"[kernel] counts max={counts.max()} caps={caps}", file=sys.stderr)

    key = (caps, D, F, MM_DTYPE)
    if key not in _KERNEL_CACHE:
        _KERNEL_CACHE[key] = build_moe_kernel(caps, D, F, dt_in)
    nc = _KERNEL_CACHE[key]

    trace = os.environ.get("KERNEL_TRACE", "0") == "1"
    res = run_bass_kernel_spmd(
        nc,
        in_maps,
        list(range(N_CORES)),
        trace=trace,
    )
    LAST_EXEC_NS = res.exec_time_ns
    LAST_RESULTS = res

    out = np.zeros((T, D), dtype=np.float32)
    for core in range(N_CORES):
        for j in range(S):
            e = slot_expert[j][core]
            ix = tok_idx[e]
            if len(ix) == 0:
                continue
            yT = res.results[core][f"yT{j}"].astype(np.float32)  # [KD, P, C]
            ye = yT[:, :, : len(ix)].transpose(2, 0, 1).reshape(len(ix), D)
            out[ix] += wts[e][:, None] * ye
    return out.astype(hidden_states.dtype), topk_ids
